# revision 7
# baseline (speedup 1.0000x reference)
"""DTW loss kernel for Trainium2 (8 NeuronCores, Bass/Tile).

Strategy
--------
reference: C[b,i,j] = ||s1[b,i]-s2[b,j]||^2 ; DTW DP over [512,512]; return
mean_b sqrt(DTW[b,-1,-1]).

Meet-in-the-middle: any monotone DTW path crosses the row-255/256 boundary
exactly once, so DTW_end = min_j F[255,j] + min(B[256,j], B[256,j+1]) where F
is the forward DP over rows 0..255 and B is the backward DP (equal to a
forward DP on the reversed sequences). Each core handles 16 batch elements *
2 directions = 32 independent half-DPs of 256 rows, stacked on 32 SBUF
partitions.

Per DP row the recurrence row[j] = c[j] + min(row[j-1], prev[j], prev[j-1])
is computed with two DVE instructions:
  m[j] = min(prev[j], prev[j-1])            (scalar_tensor_tensor, shifted APs)
  row  = tensor_tensor_scan(min, add)(m, c)  (state = min(m[j], state) + c[j])

The cost rows are produced on the PE: C[vb,i,j] = u[vb,i,:]@v[vb,j,:] with
u = [-2*s1, 1, |s1|^2], v = [s2, |s2|^2, 1] (K=18). Batched over vb via
block-diagonal weights: psum[(i_l*32+vb), j] accumulated over 8 matmul chunks
(4 vb per chunk, each vb padded to a 32-partition K-slot so all compute-engine
partition offsets stay 32-aligned). The scalar engine copies each psum row
group [32,512] to SBUF for the scans; the GPSIMD engine assembles the
block-diagonal weight tiles from a compact u tensor.
"""

import numpy as np

B = 128
L1 = 512
L2 = 512
D = 16
N_CORES = 8
PER_CORE = B // N_CORES  # 16
VB = 2 * PER_CORE  # 32 virtual batches (fwd+bwd)
R = L1 // 2  # 256 rows per half-DP
KAUG = D + 2  # 18
NCHUNK = VB // 4  # 8 matmul chunks, 4 vb each (K padded 4*32=128)
IBLK = 4  # DP rows per psum block
NBLK = R // IBLK  # 64
EIGHTH = 8  # blocks per weight-staging buffer
BIG = 1e30

_CACHE = {}


def _emit(tc, v_c, u_c, out_rows):
    import concourse.bass as bass  # noqa: F401
    from concourse import mybir

    F32 = mybir.dt.float32
    F32R = mybir.dt.float32r
    Alu = mybir.AluOpType
    nc = tc.nc

    if True:
        with (
            tc.tile_pool(name="singles", bufs=1) as singles,
            tc.tile_pool(name="crows", bufs=8) as crows,
            tc.tile_pool(name="psum", bufs=4, space="PSUM") as psum_pool,
        ):
            # --- persistent tiles ---
            rhs = [singles.tile([128, L2], F32, tag=f"rhs{g}", name=f"rhs{g}") for g in range(NCHUNK)]
            ucs = singles.tile([KAUG, VB, R], F32, tag="ucs", name="ucs")
            big = singles.tile([VB, L2], F32, tag="bigtile", name="bigtile")
            new = [singles.tile([VB, L2 + 1], F32, tag=f"new{p}", name=f"new{p}") for p in range(2)]
            mm = [singles.tile([VB, L2], F32, tag=f"m{p}", name=f"m{p}") for p in range(2)]
            # weight staging: per chunk, 2 persistent buffers [128, EIGHTH*4*32]
            wt = [
                [
                    singles.tile([128, EIGHTH, IBLK, VB], F32, tag=f"w{g}_{p}", name=f"w{g}_{p}")
                    for p in range(2)
                ]
                for g in range(NCHUNK)
            ]

            # --- prologue: memsets + input DMAs ---
            nc.vector.memset(big, BIG)
            for p in range(2):
                nc.vector.memset(new[p][:, 0:1], BIG)
            for g in range(NCHUNK):
                nc.gpsimd.memset(rhs[g], 0.0)
                for p in range(2):
                    nc.gpsimd.memset(wt[g][p], 0.0)
            # rhs rows: partitions [vl*32, vl*32+18) <- v_c[4g+vl]
            for g in range(NCHUNK):
                for vl in range(4):
                    nc.sync.dma_start(
                        out=rhs[g][vl * 32 : vl * 32 + KAUG, :].bitcast(F32R),
                        in_=v_c[4 * g + vl].bitcast(F32R),
                    )
            # compact u, split into 4 DMAs for queue parallelism
            for q in range(4):
                nc.sync.dma_start(
                    out=ucs[:, q * 8 : (q + 1) * 8, :],
                    in_=u_c[:, q * 8 : (q + 1) * 8, :],
                )

            # --- main loop over 64 psum blocks (4 DP rows each) ---
            for t in range(NBLK):
                e, tl = divmod(t, EIGHTH)
                if tl == 0:
                    # assemble block-diagonal weights for this eighth
                    i0 = e * EIGHTH * IBLK
                    for g in range(NCHUNK):
                        w = wt[g][e % 2]
                        for vl in range(4):
                            vb = 4 * g + vl
                            nc.gpsimd.tensor_copy(
                                out=w[vl * 32 : vl * 32 + KAUG, :, :, vb].bitcast(F32R),
                                in_=ucs[:, vb, i0 : i0 + EIGHTH * IBLK].rearrange(
                                    "p (a b) -> p a b", a=EIGHTH
                                ).bitcast(F32R),
                            )
                pt = psum_pool.tile([128, L2], F32, tag="pt", name=f"pt{t}")
                for g in range(NCHUNK):
                    nc.tensor.matmul(
                        out=pt,
                        lhsT=wt[g][e % 2][:, tl, :, :].bitcast(F32R),
                        rhs=rhs[g].bitcast(F32R),
                        start=(g == 0),
                        stop=(g == NCHUNK - 1),
                    )
                for il in range(IBLK):
                    r = t * IBLK + il
                    c_row = crows.tile([VB, L2], F32, tag="c_row", name=f"c_row{r}")
                    nc.scalar.copy(out=c_row, in_=pt[il * 32 : (il + 1) * 32, :])
                    nb = new[r % 2]
                    if r == 0:
                        nc.vector.tensor_tensor_scan(
                            out=nb[:, 1 : L2 + 1],
                            data0=big,
                            data1=c_row,
                            initial=0.0,
                            op0=Alu.min,
                            op1=Alu.add,
                        )
                    else:
                        pb = new[(r - 1) % 2]
                        mb = mm[r % 2]
                        nc.vector.scalar_tensor_tensor(
                            out=mb,
                            in0=pb[:, 1 : L2 + 1],
                            scalar=0.0,
                            in1=pb[:, 0:L2],
                            op0=Alu.bypass,
                            op1=Alu.min,
                        )
                        nc.vector.tensor_tensor_scan(
                            out=nb[:, 1 : L2 + 1],
                            data0=mb,
                            data1=c_row,
                            initial=BIG,
                            op0=Alu.min,
                            op1=Alu.add,
                        )
            nc.sync.dma_start(out=out_rows, in_=new[(R - 1) % 2][:, 1 : L2 + 1])


def _build():
    import concourse.bacc as bacc
    import concourse.tile as tile
    from concourse import mybir

    F32 = mybir.dt.float32
    nc = bacc.Bacc()
    v_c = nc.dram_tensor("v_c", [VB, KAUG, L2], F32, kind="ExternalInput")[:]
    u_c = nc.dram_tensor("u_c", [KAUG, VB, R], F32, kind="ExternalInput")[:]
    out_rows = nc.dram_tensor("out_rows", [VB, L2], F32, kind="ExternalOutput")[:]
    with tile.TileContext(nc) as tc:
        _emit(tc, v_c, u_c, out_rows)
    nc.compile()
    return nc


def _host_prep(s1, s2):
    """Build per-core v_c [VB,18,512] and u_c [18,VB,256] arrays."""
    s1 = np.ascontiguousarray(s1, dtype=np.float32)
    s2 = np.ascontiguousarray(s2, dtype=np.float32)
    in_maps = []
    for c in range(N_CORES):
        s1c = s1[c * PER_CORE : (c + 1) * PER_CORE]  # [16, 512, 16]
        s2c = s2[c * PER_CORE : (c + 1) * PER_CORE]
        # virtual batches: vb = bl (fwd), 16+bl (bwd)
        s1v = np.concatenate([s1c[:, :R], s1c[:, ::-1][:, :R]], axis=0)  # [32,256,16]
        s2v = np.concatenate([s2c, s2c[:, ::-1]], axis=0)  # [32,512,16]
        u = np.empty((VB, R, KAUG), np.float32)
        u[:, :, :D] = -2.0 * s1v
        u[:, :, D] = 1.0
        u[:, :, D + 1] = (s1v * s1v).sum(-1)
        v = np.empty((VB, L2, KAUG), np.float32)
        v[:, :, :D] = s2v
        v[:, :, D] = (s2v * s2v).sum(-1)
        v[:, :, D + 1] = 1.0
        in_maps.append(
            {
                "v_c": np.ascontiguousarray(v.transpose(0, 2, 1)),  # [VB,18,512]
                "u_c": np.ascontiguousarray(u.transpose(2, 0, 1)),  # [18,VB,256]
            }
        )
    return in_maps


def _combine(outs):
    """outs: list of [VB, 512] final-row arrays per core -> scalar loss."""
    vals = np.empty(B, np.float64)
    for c in range(N_CORES):
        rows = outs[c]
        for bl in range(PER_CORE):
            F = rows[bl].astype(np.float64)
            Brow = rows[PER_CORE + bl][::-1].astype(np.float64)
            Bnext = np.concatenate([Brow[1:], [np.inf]])
            vals[c * PER_CORE + bl] = np.min(F + np.minimum(Brow, Bnext))
    return np.float32(np.mean(np.sqrt(vals)))


def kernel(s1_batch, s2_batch):
    from concourse import bass_utils

    if "nc" not in _CACHE:
        _CACHE["nc"] = _build()
    nc = _CACHE["nc"]
    in_maps = _host_prep(np.asarray(s1_batch), np.asarray(s2_batch))
    res = bass_utils.run_bass_kernel_spmd(nc, in_maps, core_ids=list(range(N_CORES)))
    outs = [r["out_rows"] for r in res.results]
    return _combine(outs)


# revision 8
# speedup vs baseline: 929.7064x; 929.7064x over previous
"""DTW loss kernel for Trainium2 (8 NeuronCores, Bass/Tile).

Strategy
--------
reference: C[b,i,j] = ||s1[b,i]-s2[b,j]||^2 ; DTW DP over [512,512]; return
mean_b sqrt(DTW[b,-1,-1]).

Meet-in-the-middle: any monotone DTW path crosses the row-255/256 boundary
exactly once, so DTW_end = min_j F[255,j] + min(B[256,j], B[256,j+1]) where F
is the forward DP over rows 0..255 and B is the backward DP (equal to a
forward DP on the reversed sequences). Each core handles 16 batch elements *
2 directions = 32 independent half-DPs of 256 rows, stacked on 32 SBUF
partitions.

Per DP row the recurrence row[j] = c[j] + min(row[j-1], prev[j], prev[j-1])
is computed with two DVE instructions:
  m[j] = min(prev[j], prev[j-1])            (scalar_tensor_tensor, shifted APs)
  row  = tensor_tensor_scan(min, add)(m, c)  (state = min(m[j], state) + c[j])

The cost rows are produced on the PE: C[vb,i,j] = u[vb,i,:]@v[vb,j,:] with
u = [-2*s1, 1, |s1|^2], v = [s2, |s2|^2, 1] (K=18). Batched over vb via
block-diagonal weights: psum[(i_l*32+vb), j] accumulated over 8 matmul chunks
(4 vb per chunk, each vb padded to a 32-partition K-slot so all compute-engine
partition offsets stay 32-aligned). The scalar engine copies each psum row
group [32,512] to SBUF for the scans; the GPSIMD engine assembles the
block-diagonal weight tiles from a compact u tensor.
"""

import numpy as np

B = 128
L1 = 512
L2 = 512
D = 16
N_CORES = 8
PER_CORE = B // N_CORES  # 16
VB = 2 * PER_CORE  # 32 virtual batches (fwd+bwd)
R = L1 // 2  # 256 rows per half-DP
KAUG = D + 2  # 18
NCHUNK = VB // 4  # 8 matmul chunks, 4 vb each (K padded 4*32=128)
IBLK = 4  # DP rows per psum block
NBLK = R // IBLK  # 64
EIGHTH = 8  # blocks per weight-staging buffer
BIG = 1e30

_CACHE = {}


def _emit(tc, v_c, u_c, out_rows):
    import concourse.bass as bass  # noqa: F401
    from concourse import mybir

    F32 = mybir.dt.float32
    F32R = mybir.dt.float32r
    Alu = mybir.AluOpType
    nc = tc.nc

    if True:
        with (
            tc.tile_pool(name="singles", bufs=1) as singles,
            tc.tile_pool(name="crows", bufs=8) as crows,
            tc.tile_pool(name="psum", bufs=4, space="PSUM") as psum_pool,
        ):
            # --- persistent tiles ---
            rhs = [singles.tile([128, L2], F32, tag=f"rhs{g}", name=f"rhs{g}") for g in range(NCHUNK)]
            ucs = singles.tile([KAUG, VB, R], F32, tag="ucs", name="ucs")
            big = singles.tile([VB, L2], F32, tag="bigtile", name="bigtile")
            new = [singles.tile([VB, L2 + 1], F32, tag=f"new{p}", name=f"new{p}") for p in range(2)]
            mm = [singles.tile([VB, L2], F32, tag=f"m{p}", name=f"m{p}") for p in range(2)]
            # weight staging: per chunk, 2 persistent buffers [128, EIGHTH*4*32]
            wt = [
                [
                    singles.tile([128, EIGHTH, IBLK, VB], F32, tag=f"w{g}_{p}", name=f"w{g}_{p}")
                    for p in range(2)
                ]
                for g in range(NCHUNK)
            ]

            # --- prologue: memsets + input DMAs ---
            nc.vector.memset(big, BIG)
            for p in range(2):
                nc.vector.memset(new[p][:, 0:1], BIG)
            for g in range(NCHUNK):
                nc.gpsimd.memset(rhs[g], 0.0)
                for p in range(2):
                    nc.gpsimd.memset(wt[g][p], 0.0)
            # rhs rows: partitions [vl*32, vl*32+18) <- v_c[4g+vl]
            for g in range(NCHUNK):
                for vl in range(4):
                    nc.sync.dma_start(
                        out=rhs[g][vl * 32 : vl * 32 + KAUG, :].bitcast(F32R),
                        in_=v_c[4 * g + vl].bitcast(F32R),
                    )
            # compact u, split into 4 DMAs for queue parallelism
            for q in range(4):
                nc.sync.dma_start(
                    out=ucs[:, q * 8 : (q + 1) * 8, :],
                    in_=u_c[:, q * 8 : (q + 1) * 8, :],
                )

            # --- main loop over 64 psum blocks (4 DP rows each) ---
            for t in range(NBLK):
                e, tl = divmod(t, EIGHTH)
                if tl == 0:
                    # assemble block-diagonal weights for this eighth
                    i0 = e * EIGHTH * IBLK
                    for g in range(NCHUNK):
                        w = wt[g][e % 2]
                        for vl in range(4):
                            vb = 4 * g + vl
                            nc.gpsimd.tensor_copy(
                                out=w[vl * 32 : vl * 32 + KAUG, :, :, vb].bitcast(F32R),
                                in_=ucs[:, vb, i0 : i0 + EIGHTH * IBLK].rearrange(
                                    "p (a b) -> p a b", a=EIGHTH
                                ).bitcast(F32R),
                            )
                pt = psum_pool.tile([128, L2], F32, tag="pt", name=f"pt{t}")
                for g in range(NCHUNK):
                    nc.tensor.matmul(
                        out=pt,
                        lhsT=wt[g][e % 2][:, tl, :, :].bitcast(F32R),
                        rhs=rhs[g].bitcast(F32R),
                        start=(g == 0),
                        stop=(g == NCHUNK - 1),
                    )
                for il in range(IBLK):
                    r = t * IBLK + il
                    c_row = crows.tile([VB, L2], F32, tag="c_row", name=f"c_row{r}")
                    nc.scalar.copy(out=c_row, in_=pt[il * 32 : (il + 1) * 32, :])
                    nb = new[r % 2]
                    if r == 0:
                        nc.vector.tensor_tensor_scan(
                            out=nb[:, 1 : L2 + 1],
                            data0=big,
                            data1=c_row,
                            initial=0.0,
                            op0=Alu.min,
                            op1=Alu.add,
                        )
                    else:
                        pb = new[(r - 1) % 2]
                        mb = mm[r % 2]
                        nc.vector.scalar_tensor_tensor(
                            out=mb,
                            in0=pb[:, 1 : L2 + 1],
                            scalar=0.0,
                            in1=pb[:, 0:L2],
                            op0=Alu.bypass,
                            op1=Alu.min,
                        )
                        nc.vector.tensor_tensor_scan(
                            out=nb[:, 1 : L2 + 1],
                            data0=mb,
                            data1=c_row,
                            initial=BIG,
                            op0=Alu.min,
                            op1=Alu.add,
                        )
            nc.sync.dma_start(out=out_rows, in_=new[(R - 1) % 2][:, 1 : L2 + 1])


def _build():
    import concourse.bacc as bacc
    import concourse.tile as tile
    from concourse import mybir

    F32 = mybir.dt.float32
    nc = bacc.Bacc()
    v_c = nc.dram_tensor("v_c", [VB, KAUG, L2], F32, kind="ExternalInput")[:]
    u_c = nc.dram_tensor("u_c", [KAUG, VB, R], F32, kind="ExternalInput")[:]
    out_rows = nc.dram_tensor("out_rows", [VB, L2], F32, kind="ExternalOutput")[:]
    with tile.TileContext(nc) as tc:
        _emit(tc, v_c, u_c, out_rows)
    nc.compile()
    return nc


def _host_prep(s1, s2):
    """Build per-core v_c [VB,18,512] and u_c [18,VB,256] arrays."""
    s1 = np.ascontiguousarray(s1, dtype=np.float32)
    s2 = np.ascontiguousarray(s2, dtype=np.float32)
    in_maps = []
    for c in range(N_CORES):
        s1c = s1[c * PER_CORE : (c + 1) * PER_CORE]  # [16, 512, 16]
        s2c = s2[c * PER_CORE : (c + 1) * PER_CORE]
        # virtual batches: vb = bl (fwd), 16+bl (bwd)
        s1v = np.concatenate([s1c[:, :R], s1c[:, ::-1][:, :R]], axis=0)  # [32,256,16]
        s2v = np.concatenate([s2c, s2c[:, ::-1]], axis=0)  # [32,512,16]
        u = np.empty((VB, R, KAUG), np.float32)
        u[:, :, :D] = -2.0 * s1v
        u[:, :, D] = 1.0
        u[:, :, D + 1] = (s1v * s1v).sum(-1)
        v = np.empty((VB, L2, KAUG), np.float32)
        v[:, :, :D] = s2v
        v[:, :, D] = (s2v * s2v).sum(-1)
        v[:, :, D + 1] = 1.0
        in_maps.append(
            {
                "v_c": np.ascontiguousarray(v.transpose(0, 2, 1)),  # [VB,18,512]
                "u_c": np.ascontiguousarray(u.transpose(2, 0, 1)),  # [18,VB,256]
            }
        )
    return in_maps


def _combine(outs):
    """outs: list of [VB, 512] final-row arrays per core -> scalar loss."""
    vals = np.empty(B, np.float64)
    for c in range(N_CORES):
        rows = outs[c]
        for bl in range(PER_CORE):
            F = rows[bl].astype(np.float64)
            Brow = rows[PER_CORE + bl][::-1].astype(np.float64)
            Bnext = np.concatenate([Brow[1:], [np.inf]])
            vals[c * PER_CORE + bl] = np.min(F + np.minimum(Brow, Bnext))
    return np.float32(np.mean(np.sqrt(vals)))


def kernel(s1_batch, s2_batch):
    from concourse import bass_utils

    if "nc" not in _CACHE:
        _CACHE["nc"] = _build()
    nc = _CACHE["nc"]
    in_maps = _host_prep(np.asarray(s1_batch), np.asarray(s2_batch))
    kw = {}
    if _CACHE.get("trace"):
        kw = dict(trace=True, trace_cores=_CACHE.get("trace_cores", [0]),
                  tmpdir=_CACHE.get("tmpdir"))
    res = bass_utils.run_bass_kernel_spmd(
        nc, in_maps, core_ids=list(range(N_CORES)), **kw
    )
    if res.exec_time_ns is not None:
        _CACHE["exec_time_ns"] = res.exec_time_ns
    _CACHE["last_results"] = res
    outs = [r["out_rows"] for r in res.results]
    return _combine(outs)


# revision 9
# speedup vs baseline: 1232.3223x; 1.3255x over previous
"""DTW loss kernel for Trainium2 (8 NeuronCores, Bass/Tile).

Strategy
--------
reference: C[b,i,j] = ||s1[b,i]-s2[b,j]||^2 ; DTW DP over [512,512]; return
mean_b sqrt(DTW[b,-1,-1]).

Meet-in-the-middle: any monotone DTW path crosses the row-255/256 boundary
exactly once, so DTW_end = min_j F[255,j] + min(B[256,j], B[256,j+1]) where F
is the forward DP over rows 0..255 and B the backward DP (a forward DP on the
reversed sequences). Each core handles 16 batch elements * 2 directions = 32
independent half-DPs ("virtual batches", vb) of 256 rows.

DP rows are computed with tensor_tensor_scan (state = min(m[j], state) + c[j])
plus one scalar_tensor_tensor for m[j] = min(prev[j], prev[j-1]). To shorten
the serial free-dim, a 2-block wavefront runs on 64 partitions = (q, vb),
q in {0,1}: at superstep s lane (0,vb) scans row s cols [0,256) and lane
(1,vb) scans row s-1 cols [256,512). Block carries ride in column 0 of the
row tile: one [32,1] copy per superstep moves lane-q0's tail both into the
scan's per-partition `initial` AP and into the m-prep's j-1 edge slot.

The cost rows are made on the PE in bf16: C[vb,i,j] = u[vb,i,:]@v[vb,j,:]
with u = [-2*s1, 1, |s1|^2], v = [s2, |s2|^2, 1] (K=18), batched over vb via
block-diagonal weights (8 chunks of 4 vb, each vb padded to a 32-partition
K-slot so compute-engine partition offsets stay 32-aligned). GPSIMD casts the
compact f32 u into the bf16 weight tiles; the scalar engine gathers psum
[32,256] pieces into the wavefront layout.
"""

import numpy as np

B = 128
L1 = 512
L2 = 512
D = 16
N_CORES = 8
PER_CORE = B // N_CORES  # 16
VB = 2 * PER_CORE  # 32 virtual batches (fwd+bwd)
R = L1 // 2  # 256 rows per half-DP
KAUG = D + 2  # 18
NCHUNK = VB // 4  # 8 matmul chunks, 4 vb each (K padded 4*32=128)
IBLK = 4  # DP rows per psum block
NBLK = R // IBLK  # 64
EIGHTH = 8  # psum blocks per weight-staging buffer
W = L2 // 2  # 256: wavefront block width
NSS = R + 1  # 257 supersteps
BIG = 1e30

_CACHE = {}


def _emit(tc, v_c, u_c, out_rows):
    import concourse.bass as bass  # noqa: F401
    from concourse import mybir

    F32 = mybir.dt.float32
    Alu = mybir.AluOpType
    nc = tc.nc

    with (
        tc.tile_pool(name="singles", bufs=1) as singles,
        tc.tile_pool(name="psum", bufs=4, space="PSUM") as psum_pool,
    ):
        BF16 = mybir.dt.bfloat16
        # --- persistent tiles ---
        rhs = [singles.tile([128, L2], BF16, tag=f"rhs{g}", name=f"rhs{g}") for g in range(NCHUNK)]
        ucs = singles.tile([KAUG, VB, R], F32, tag="ucs", name="ucs")
        bigm = singles.tile([2 * VB, W], F32, tag="bigm", name="bigm")
        init0 = singles.tile([2 * VB, 1], F32, tag="init0", name="init0")
        new = [singles.tile([2 * VB, W + 1], F32, tag=f"new{p}", name=f"new{p}") for p in range(3)]
        mm = [singles.tile([2 * VB, W], F32, tag=f"m{p}", name=f"m{p}") for p in range(2)]
        cc = [singles.tile([2 * VB, W], F32, tag=f"c{p}", name=f"c{p}") for p in range(4)]
        wt = [
            [
                singles.tile([128, EIGHTH, IBLK, VB], BF16, tag=f"w{g}_{p}", name=f"w{g}_{p}")
                for p in range(2)
            ]
            for g in range(NCHUNK)
        ]

        # --- prologue ---
        nc.vector.memset(bigm, BIG)
        nc.vector.memset(init0, 0.0)
        for p in range(3):
            nc.vector.memset(new[p][0:VB, 0:1], BIG)
        for p in range(4):
            nc.vector.memset(cc[p], 0.0)
        for g in range(NCHUNK):
            nc.gpsimd.memset(rhs[g], 0.0)
            for p in range(2):
                nc.gpsimd.memset(wt[g][p], 0.0)
        for g in range(NCHUNK):
            for vl in range(4):
                nc.sync.dma_start(
                    out=rhs[g][vl * 32 : vl * 32 + KAUG, :],
                    in_=v_c[4 * g + vl],
                )
        for q in range(4):
            nc.sync.dma_start(
                out=ucs[:, q * 8 : (q + 1) * 8, :],
                in_=u_c[:, q * 8 : (q + 1) * 8, :],
            )

        psum_tiles = {}

        def emit_block(t):
            e, tl = divmod(t, EIGHTH)
            if tl == 0:
                i0 = e * EIGHTH * IBLK
                for g in range(NCHUNK):
                    w = wt[g][e % 2]
                    for vl in range(4):
                        vb = 4 * g + vl
                        nc.gpsimd.tensor_copy(
                            out=w[vl * 32 : vl * 32 + KAUG, :, :, vb],
                            in_=ucs[:, vb, i0 : i0 + EIGHTH * IBLK].rearrange(
                                "p (a b) -> p a b", a=EIGHTH
                            ),
                        )
            pt = psum_pool.tile([128, L2], F32, tag="pt", name=f"pt{t}")
            for g in range(NCHUNK):
                nc.tensor.matmul(
                    out=pt,
                    lhsT=wt[g][e % 2][:, tl, :, :],
                    rhs=rhs[g],
                    start=(g == 0),
                    stop=(g == NCHUNK - 1),
                )
            psum_tiles[t] = pt

        # --- wavefront: superstep s: lane q=0 -> row s cols [0,W);
        #     lane q=1 -> row s-1 cols [W,2W) ---
        for s in range(NSS):
            if s % IBLK == 0 and s // IBLK < NBLK:
                emit_block(s // IBLK)
            c_s = cc[s % 4]
            if s < R:
                pt = psum_tiles[s // IBLK]
                nc.scalar.copy(
                    out=c_s[0:VB, :],
                    in_=pt[32 * (s % IBLK) : 32 * (s % IBLK) + 32, 0:W],
                )
            if s >= 1:
                ptm = psum_tiles[(s - 1) // IBLK]
                nc.scalar.copy(
                    out=c_s[VB : 2 * VB, :],
                    in_=ptm[32 * ((s - 1) % IBLK) : 32 * ((s - 1) % IBLK) + 32, W:L2],
                )
            nb = new[s % 3]
            if s == 0:
                d0 = bigm
                ini = init0[:, 0:1]
            else:
                pb = new[(s - 1) % 3]
                nc.vector.tensor_copy(out=nb[VB : 2 * VB, 0:1], in_=pb[0:VB, W : W + 1])
                mb = mm[s % 2]
                if s == 1:
                    nc.vector.scalar_tensor_tensor(
                        out=mb[0:VB, :], in0=pb[0:VB, 1 : W + 1], scalar=0.0,
                        in1=pb[0:VB, 0:W], op0=Alu.bypass, op1=Alu.min,
                    )
                    nc.vector.memset(mb[VB : 2 * VB, :], BIG)
                else:
                    nc.vector.scalar_tensor_tensor(
                        out=mb, in0=pb[:, 1 : W + 1], scalar=0.0,
                        in1=pb[:, 0:W], op0=Alu.bypass, op1=Alu.min,
                    )
                d0 = mb
                ini = nb[:, 0:1]
            nc.vector.tensor_tensor_scan(
                out=nb[:, 1 : W + 1], data0=d0, data1=c_s, initial=ini,
                op0=Alu.min, op1=Alu.add,
            )
        nc.sync.dma_start(
            out=out_rows[:, 0:W], in_=new[(R - 1) % 3][0:VB, 1 : W + 1]
        )
        nc.sync.dma_start(
            out=out_rows[:, W:L2], in_=new[R % 3][VB : 2 * VB, 1 : W + 1]
        )


def _build():
    import concourse.bacc as bacc
    import concourse.tile as tile
    from concourse import mybir

    F32 = mybir.dt.float32
    BF16 = mybir.dt.bfloat16
    nc = bacc.Bacc()
    v_c = nc.dram_tensor("v_c", [VB, KAUG, L2], BF16, kind="ExternalInput")[:]
    u_c = nc.dram_tensor("u_c", [KAUG, VB, R], F32, kind="ExternalInput")[:]
    out_rows = nc.dram_tensor("out_rows", [VB, L2], F32, kind="ExternalOutput")[:]
    with tile.TileContext(nc) as tc:
        _emit(tc, v_c, u_c, out_rows)
    nc.compile()
    return nc


def _host_prep(s1, s2):
    """Build per-core v_c [VB,18,512] (bf16) and u_c [18,VB,256] (f32)."""
    import ml_dtypes

    s1 = np.ascontiguousarray(s1, dtype=np.float32)
    s2 = np.ascontiguousarray(s2, dtype=np.float32)
    in_maps = []
    for c in range(N_CORES):
        s1c = s1[c * PER_CORE : (c + 1) * PER_CORE]  # [16, 512, 16]
        s2c = s2[c * PER_CORE : (c + 1) * PER_CORE]
        s1v = np.concatenate([s1c[:, :R], s1c[:, ::-1][:, :R]], axis=0)  # [32,256,16]
        s2v = np.concatenate([s2c, s2c[:, ::-1]], axis=0)  # [32,512,16]
        u = np.empty((VB, R, KAUG), np.float32)
        u[:, :, :D] = -2.0 * s1v
        u[:, :, D] = 1.0
        u[:, :, D + 1] = (s1v * s1v).sum(-1)
        v = np.empty((VB, L2, KAUG), np.float32)
        v[:, :, :D] = s2v
        v[:, :, D] = (s2v * s2v).sum(-1)
        v[:, :, D + 1] = 1.0
        in_maps.append(
            {
                "v_c": np.ascontiguousarray(
                    v.transpose(0, 2, 1).astype(ml_dtypes.bfloat16)
                ),  # [VB,18,512] bf16
                "u_c": np.ascontiguousarray(u.transpose(2, 0, 1)),  # [18,VB,256] f32
            }
        )
    return in_maps


def _combine(outs):
    """outs: list of [VB, 512] final-row arrays per core -> scalar loss."""
    vals = np.empty(B, np.float64)
    for c in range(N_CORES):
        rows = outs[c]
        for bl in range(PER_CORE):
            F = rows[bl].astype(np.float64)
            Brow = rows[PER_CORE + bl][::-1].astype(np.float64)
            Bnext = np.concatenate([Brow[1:], [np.inf]])
            vals[c * PER_CORE + bl] = np.min(F + np.minimum(Brow, Bnext))
    return np.float32(np.mean(np.sqrt(vals)))


def kernel(s1_batch, s2_batch):
    from concourse import bass_utils

    if "nc" not in _CACHE:
        _CACHE["nc"] = _build()
    nc = _CACHE["nc"]
    in_maps = _host_prep(np.asarray(s1_batch), np.asarray(s2_batch))
    kw = {}
    if _CACHE.get("trace"):
        kw = dict(trace=True, trace_cores=_CACHE.get("trace_cores", [0]),
                  tmpdir=_CACHE.get("tmpdir"))
    res = bass_utils.run_bass_kernel_spmd(
        nc, in_maps, core_ids=list(range(N_CORES)), **kw
    )
    if res.exec_time_ns is not None:
        _CACHE["exec_time_ns"] = res.exec_time_ns
    _CACHE["last_results"] = res
    outs = [r["out_rows"] for r in res.results]
    return _combine(outs)


# revision 10
# speedup vs baseline: 1443.1105x; 1.1710x over previous
"""DTW loss kernel for Trainium2 (8 NeuronCores, Bass/Tile).

Strategy
--------
reference: C[b,i,j] = ||s1[b,i]-s2[b,j]||^2 ; DTW DP over [512,512]; return
mean_b sqrt(DTW[b,-1,-1]).

Meet-in-the-middle: any monotone DTW path crosses the row-255/256 boundary
exactly once, so DTW_end = min_j F[255,j] + min(B[256,j], B[256,j+1]) where F
is the forward DP over rows 0..255 and B the backward DP (a forward DP on the
reversed sequences). Each core handles 16 batch elements * 2 directions = 32
independent half-DPs ("virtual batches", vb) of 256 rows.

DP rows are computed with tensor_tensor_scan (state = min(m[j], state) + c[j])
plus one scalar_tensor_tensor for m[j] = min(prev[j], prev[j-1]). To shorten
the serial free-dim, a 2-block wavefront runs on 64 partitions = (q, vb),
q in {0,1}: at superstep s lane (0,vb) scans row s cols [0,256) and lane
(1,vb) scans row s-1 cols [256,512). Block carries ride in column 0 of the
row tile: one [32,1] copy per superstep moves lane-q0's tail both into the
scan's per-partition `initial` AP and into the m-prep's j-1 edge slot.

The cost rows are made on the PE in bf16: C[vb,i,j] = u[vb,i,:]@v[vb,j,:]
with u = [-2*s1, 1, |s1|^2], v = [s2, |s2|^2, 1] (K=18), batched over vb via
block-diagonal weights (8 chunks of 4 vb, each vb padded to a 32-partition
K-slot so compute-engine partition offsets stay 32-aligned). GPSIMD casts the
compact f32 u into the bf16 weight tiles; the scalar engine gathers psum
[32,256] pieces into the wavefront layout.
"""

import numpy as np

B = 128
L1 = 512
L2 = 512
D = 16
N_CORES = 8
PER_CORE = B // N_CORES  # 16
VB = 2 * PER_CORE  # 32 virtual batches (fwd+bwd)
R = L1 // 2  # 256 rows per half-DP
KAUG = D + 2  # 18
NCHUNK = 5  # matmul chunks of up to 7 vb, K rows = 7*18 = 126 (unpadded)
KCH = 126  # K rows per chunk
IBLK = 4  # DP rows per psum block
NBLK = R // IBLK  # 64
EIGHTH = 8  # psum blocks per weight-staging buffer
W = L2 // 2  # 256: wavefront block width
NSS = R + 1  # 257 supersteps
BIG = 1e30

_CACHE = {}


def _emit(tc, v_c, w_c, out_rows):
    import concourse.bass as bass  # noqa: F401
    from concourse import mybir

    F32 = mybir.dt.float32
    Alu = mybir.AluOpType
    nc = tc.nc

    with (
        tc.tile_pool(name="singles", bufs=1) as singles,
        tc.tile_pool(name="wpool", bufs=12) as wpool,
        tc.tile_pool(name="psum", bufs=4, space="PSUM") as psum_pool,
    ):
        BF16 = mybir.dt.bfloat16
        # --- persistent tiles ---
        rhs = [singles.tile([KCH, L2], BF16, tag=f"rhs{g}", name=f"rhs{g}") for g in range(NCHUNK)]
        bigm = singles.tile([2 * VB, W], F32, tag="bigm", name="bigm")
        init0 = singles.tile([2 * VB, 1], F32, tag="init0", name="init0")
        new = [singles.tile([2 * VB, W + 1], F32, tag=f"new{p}", name=f"new{p}") for p in range(3)]
        mm = [singles.tile([2 * VB, W], F32, tag=f"m{p}", name=f"m{p}") for p in range(2)]
        cc = [singles.tile([2 * VB, W], F32, tag=f"c{p}", name=f"c{p}") for p in range(4)]

        # --- prologue ---
        nc.vector.memset(bigm, BIG)
        nc.vector.memset(init0, 0.0)
        for p in range(3):
            nc.vector.memset(new[p][0:VB, 0:1], BIG)
        for p in range(4):
            nc.vector.memset(cc[p], 0.0)
        for g in range(NCHUNK):
            nc.sync.dma_start(out=rhs[g], in_=v_c[g])

        psum_tiles = {}

        def emit_block(t):
            pt = psum_pool.tile([128, L2], F32, tag="pt", name=f"pt{t}")
            for g in range(NCHUNK):
                w = wpool.tile([KCH, 128], BF16, tag="w", name=f"w{t}_{g}")
                nc.sync.dma_start(out=w, in_=w_c[t, g])
                nc.tensor.matmul(
                    out=pt,
                    lhsT=w,
                    rhs=rhs[g],
                    start=(g == 0),
                    stop=(g == NCHUNK - 1),
                )
            psum_tiles[t] = pt

        # --- wavefront: superstep s: lane q=0 -> row s cols [0,W);
        #     lane q=1 -> row s-1 cols [W,2W) ---
        for s in range(NSS):
            if s % IBLK == 0 and s // IBLK < NBLK:
                emit_block(s // IBLK)
            c_s = cc[s % 4]
            if s < R:
                pt = psum_tiles[s // IBLK]
                nc.scalar.copy(
                    out=c_s[0:VB, :],
                    in_=pt[32 * (s % IBLK) : 32 * (s % IBLK) + 32, 0:W],
                )
            if s >= 1:
                ptm = psum_tiles[(s - 1) // IBLK]
                nc.scalar.copy(
                    out=c_s[VB : 2 * VB, :],
                    in_=ptm[32 * ((s - 1) % IBLK) : 32 * ((s - 1) % IBLK) + 32, W:L2],
                )
            nb = new[s % 3]
            if s == 0:
                d0 = bigm
                ini = init0[:, 0:1]
            else:
                pb = new[(s - 1) % 3]
                nc.scalar.copy(out=nb[VB : 2 * VB, 0:1], in_=pb[0:VB, W : W + 1])
                mb = mm[s % 2]
                if s == 1:
                    nc.vector.scalar_tensor_tensor(
                        out=mb[0:VB, :], in0=pb[0:VB, 1 : W + 1], scalar=0.0,
                        in1=pb[0:VB, 0:W], op0=Alu.bypass, op1=Alu.min,
                    )
                    nc.vector.memset(mb[VB : 2 * VB, :], BIG)
                else:
                    nc.vector.scalar_tensor_tensor(
                        out=mb, in0=pb[:, 1 : W + 1], scalar=0.0,
                        in1=pb[:, 0:W], op0=Alu.bypass, op1=Alu.min,
                    )
                d0 = mb
                ini = nb[:, 0:1]
            nc.vector.tensor_tensor_scan(
                out=nb[:, 1 : W + 1], data0=d0, data1=c_s, initial=ini,
                op0=Alu.min, op1=Alu.add,
            )
        nc.sync.dma_start(
            out=out_rows[:, 0:W], in_=new[(R - 1) % 3][0:VB, 1 : W + 1]
        )
        nc.sync.dma_start(
            out=out_rows[:, W:L2], in_=new[R % 3][VB : 2 * VB, 1 : W + 1]
        )


def _build():
    import concourse.bacc as bacc
    import concourse.tile as tile
    from concourse import mybir

    F32 = mybir.dt.float32
    BF16 = mybir.dt.bfloat16
    nc = bacc.Bacc()
    v_c = nc.dram_tensor("v_c", [NCHUNK, KCH, L2], BF16, kind="ExternalInput")[:]
    w_c = nc.dram_tensor("w_c", [NBLK, NCHUNK, KCH, 128], BF16, kind="ExternalInput")[:]
    out_rows = nc.dram_tensor("out_rows", [VB, L2], F32, kind="ExternalOutput")[:]
    with tile.TileContext(nc) as tc:
        _emit(tc, v_c, w_c, out_rows)
    nc.compile()
    return nc


def _host_prep(s1, s2):
    """Build per-core v_c [5,126,512] (bf16 rhs chunks) and the full
    block-diagonal weight tensor w_c [64,5,126,128] (bf16)."""
    import ml_dtypes

    BF = ml_dtypes.bfloat16
    s1 = np.ascontiguousarray(s1, dtype=np.float32)
    s2 = np.ascontiguousarray(s2, dtype=np.float32)
    in_maps = []
    for c in range(N_CORES):
        s1c = s1[c * PER_CORE : (c + 1) * PER_CORE]  # [16, 512, 16]
        s2c = s2[c * PER_CORE : (c + 1) * PER_CORE]
        s1v = np.concatenate([s1c[:, :R], s1c[:, ::-1][:, :R]], axis=0)  # [32,256,16]
        s2v = np.concatenate([s2c, s2c[:, ::-1]], axis=0)  # [32,512,16]
        u = np.empty((VB, R, KAUG), np.float32)
        u[:, :, :D] = -2.0 * s1v
        u[:, :, D] = 1.0
        u[:, :, D + 1] = (s1v * s1v).sum(-1)
        v = np.empty((VB, L2, KAUG), np.float32)
        v[:, :, :D] = s2v
        v[:, :, D] = (s2v * s2v).sum(-1)
        v[:, :, D + 1] = 1.0
        u = u.astype(BF)
        vch = np.zeros((NCHUNK, KCH, L2), BF)
        wch = np.zeros((NBLK, NCHUNK, KCH, 128), BF)
        for g in range(NCHUNK):
            for vl in range(min(7, VB - 7 * g)):
                vb = 7 * g + vl
                vch[g, vl * KAUG : (vl + 1) * KAUG, :] = v[vb].T
                # w[t, g, vl*18+d, il*32+vb] = u[vb, 4t+il, d]
                wch[:, g, vl * KAUG : (vl + 1) * KAUG, vb::VB] = (
                    u[vb].reshape(NBLK, IBLK, KAUG).transpose(0, 2, 1)
                )
        in_maps.append(
            {
                "v_c": vch,
                "w_c": wch,
            }
        )
    return in_maps


def _combine(outs):
    """outs: list of [VB, 512] final-row arrays per core -> scalar loss."""
    vals = np.empty(B, np.float64)
    for c in range(N_CORES):
        rows = outs[c]
        for bl in range(PER_CORE):
            F = rows[bl].astype(np.float64)
            Brow = rows[PER_CORE + bl][::-1].astype(np.float64)
            Bnext = np.concatenate([Brow[1:], [np.inf]])
            vals[c * PER_CORE + bl] = np.min(F + np.minimum(Brow, Bnext))
    return np.float32(np.mean(np.sqrt(vals)))


def kernel(s1_batch, s2_batch):
    from concourse import bass_utils

    if "nc" not in _CACHE:
        _CACHE["nc"] = _build()
    nc = _CACHE["nc"]
    in_maps = _host_prep(np.asarray(s1_batch), np.asarray(s2_batch))
    kw = {}
    if _CACHE.get("trace"):
        kw = dict(trace=True, trace_cores=_CACHE.get("trace_cores", [0]),
                  tmpdir=_CACHE.get("tmpdir"))
    res = bass_utils.run_bass_kernel_spmd(
        nc, in_maps, core_ids=list(range(N_CORES)), **kw
    )
    if res.exec_time_ns is not None:
        _CACHE["exec_time_ns"] = res.exec_time_ns
    _CACHE["last_results"] = res
    outs = [r["out_rows"] for r in res.results]
    return _combine(outs)
